# revision 2
# baseline (speedup 1.0000x reference)
"""Trainium2 Bass kernel for nn_Conv2d (B=32, Cin=Cout=64, H=W=112, 3x3, pad 1).

Strategy:
- Data-parallel: 32 images / 8 cores = 4 images per core; weights/bias replicated.
- Per core: process 2 image-PAIRS. Image A lives on SBUF partitions 0-63,
  image B on partitions 64-127, each zero-padded to 114x114 and flattened.
- Conv = 9 accumulating PE matmuls per 512-pixel chunk: for tap (r,c) the
  stationary lhsT is a 128x128 block-diagonal tile diag(w_rc^T, w_rc^T) so the
  two images convolve independently; the moving rhs is the padded image buffer
  at free-dim offset r*114+c. PSUM (fp32) accumulates all 9 taps.
- Epilogue: VectorE tensor_scalar_add(psum + per-partition bias) -> SBUF staging,
  then one strided DMA per image drops the pad columns.
- dtype float32r: fp32 storage, fast PE mode (~1 cycle/row at N=512),
  measured ~1.5e-4 max relative error end-to-end.
"""
import numpy as np

B, CIN, COUT, H, W = 32, 64, 64, 112, 112
N_CORES = 8
IPC = B // N_CORES          # images per core = 4
NPAIR = IPC // 2            # image pairs per core = 2
Wp = W + 2                  # padded width 114
Hp = H + 2                  # padded height 114
LVAL = H * Wp               # output b-domain length 12768
CH = 512                    # chunk size (one PSUM bank of fp32)
NCHUNK = (LVAL + CH - 1) // CH   # 25
LB = (NCHUNK + 1) * CH      # padded-image buffer length 13312 (tail margin)

_CACHE = {}


def _build_module():
    import concourse.tile as tile
    from concourse import bacc, mybir
    from concourse.bass_interp import get_hw_module

    f32 = mybir.dt.float32
    f32r = mybir.dt.float32r

    nc = bacc.Bacc("TRN2", target_bir_lowering=False, debug=False,
                   enable_asserts=False, num_devices=N_CORES)
    x_ap = nc.dram_tensor("x", [IPC, CIN, H, W], f32r, kind="ExternalInput").ap()
    wt_ap = nc.dram_tensor("wt", [9, 128, 128], f32r, kind="ExternalInput").ap()
    b_ap = nc.dram_tensor("bias2", [128, 1], f32, kind="ExternalInput").ap()
    y_ap = nc.dram_tensor("y", [IPC, COUT, H, W], f32, kind="ExternalOutput").ap()

    with tile.TileContext(nc) as tc:
        with (
            tc.tile_pool(name="const", bufs=1) as cp,
            tc.tile_pool(name="x2", bufs=2) as xp,
            tc.tile_pool(name="oimg", bufs=1) as op,
            tc.tile_pool(name="psum", bufs=6, space="PSUM") as pp,
        ):
            w_sb = cp.tile([128, 9 * 128], f32r)
            nc.sync.dma_start(w_sb[:].rearrange("k (t m) -> k t m", t=9),
                              wt_ap.rearrange("t k m -> k t m"))
            bias_sb = cp.tile([128, 1], f32)
            nc.sync.dma_start(bias_sb[:], b_ap[:])

            for p in range(NPAIR):
                x2 = xp.tile([128, LB], f32r)
                # zero the pad borders (rest is overwritten by the image DMA);
                # memset doesn't accept f32r so bitcast the views to f32
                nc.vector.memset(x2[:, 0:Wp].bitcast(f32), 0.0)          # top pad row
                nc.vector.memset(x2[:, (Hp - 1) * Wp:LB].bitcast(f32), 0.0)  # bottom+tail
                interior = x2[:, Wp:Wp + H * Wp].rearrange("p (h w) -> p h w", w=Wp)
                nc.vector.memset(interior[:, :, 0:1].bitcast(f32), 0.0)  # left pad col
                nc.vector.memset(interior[:, :, Wp - 1:Wp].bitcast(f32), 0.0)  # right pad
                for h in range(2):
                    img = 2 * p + h
                    dst = x2[64 * h:64 * (h + 1), Wp + 1:Wp + 1 + H * Wp]
                    dst = dst.rearrange("p (h w) -> p h w", w=Wp)[:, :, 0:W]
                    nc.sync.dma_start(dst, x_ap[img])

                oimg = op.tile([128, LVAL], f32)
                for c in range(NCHUNK):
                    s = c * CH
                    ps = pp.tile([128, CH], f32)
                    for t in range(9):
                        r, cc = divmod(t, 3)
                        off = r * Wp + cc
                        nc.tensor.matmul(ps[:], w_sb[:, t * 128:(t + 1) * 128],
                                         x2[:, s + off:s + off + CH],
                                         start=(t == 0), stop=(t == 8))
                    ev = min(CH, LVAL - s)
                    nc.vector.tensor_scalar_add(oimg[:, s:s + ev], ps[:, 0:ev],
                                                bias_sb[:])
                for h in range(2):
                    img = 2 * p + h
                    src = oimg[64 * h:64 * (h + 1), :]
                    src = src.rearrange("p (h w) -> p h w", w=Wp)[:, :, 0:W]
                    nc.sync.dma_start(y_ap[img], src)

    nc.compile()
    nc.m = get_hw_module(nc.m)
    return nc


def _get_module():
    if "nc" not in _CACHE:
        _CACHE["nc"] = _build_module()
    return _CACHE["nc"]


def _make_in_maps(x, weight, bias):
    wt = np.zeros((9, 128, 128), np.float32)
    for t in range(9):
        r, cc = divmod(t, 3)
        wT = np.ascontiguousarray(weight[:, :, r, cc].T)  # [cin, cout]
        wt[t, :64, :64] = wT
        wt[t, 64:, 64:] = wT
    bias2 = np.tile(np.asarray(bias, np.float32).reshape(COUT, 1), (2, 1))
    x = np.asarray(x, np.float32)
    return [{"x": np.ascontiguousarray(x[c * IPC:(c + 1) * IPC]),
             "wt": wt, "bias2": bias2} for c in range(N_CORES)]


def _run(in_maps, trace=False):
    from concourse import bass_utils
    nc = _get_module()
    return bass_utils.run_bass_kernel_spmd(
        nc, in_maps, core_ids=list(range(N_CORES)), trace=trace)


def kernel(x, weight, bias):
    res = _run(_make_in_maps(x, weight, bias), trace=False)
    return np.concatenate([res.results[c]["y"] for c in range(N_CORES)], axis=0)


# revision 4
# speedup vs baseline: 1.5181x; 1.5181x over previous
"""Trainium2 Bass kernel for nn_Conv2d (B=32, Cin=Cout=64, H=W=112, 3x3, pad 1).

Strategy:
- Data-parallel: 32 images / 8 cores = 4 images per core; weights/bias replicated.
- Per core: process 2 image-PAIRS. Image A lives on SBUF partitions 0-63,
  image B on partitions 64-127, each zero-padded to 114x114 and flattened.
- Conv = 9 accumulating PE matmuls per 512-pixel chunk: for tap (r,c) the
  stationary lhsT is a 128x128 block-diagonal tile diag(w_rc^T, w_rc^T) so the
  two images convolve independently; the moving rhs is the padded image buffer
  at free-dim offset r*114+c. PSUM (fp32) accumulates all 9 taps.
- Epilogue: VectorE tensor_scalar_add(psum + per-partition bias) -> SBUF staging,
  then one strided DMA per image drops the pad columns.
- dtype float32r: fp32 storage, fast PE mode (~1 cycle/row at N=512),
  measured ~1.5e-4 max relative error end-to-end.
"""
import numpy as np

B, CIN, COUT, H, W = 32, 64, 64, 112, 112
N_CORES = 8
IPC = B // N_CORES          # images per core = 4
NPAIR = IPC // 2            # image pairs per core = 2
Wp = W + 2                  # padded width 114
Hp = H + 2                  # padded height 114
LVAL = H * Wp               # output b-domain length 12768
CH = 512                    # chunk size (one PSUM bank of fp32)
NCHUNK = (LVAL + CH - 1) // CH   # 25
LB = (NCHUNK + 1) * CH      # padded-image buffer length 13312 (tail margin)

_CACHE = {}


def _build_module():
    import concourse.tile as tile
    from concourse import bacc, mybir
    from concourse.bass_interp import get_hw_module

    f32 = mybir.dt.float32
    f32r = mybir.dt.float32r

    nc = bacc.Bacc("TRN2", target_bir_lowering=False, debug=False,
                   enable_asserts=False, num_devices=N_CORES)
    x_ap = nc.dram_tensor("x", [IPC, CIN, H, W], f32r, kind="ExternalInput").ap()
    wt_ap = nc.dram_tensor("wt", [9, 128, 128], f32r, kind="ExternalInput").ap()
    b_ap = nc.dram_tensor("bias2", [128, 1], f32, kind="ExternalInput").ap()
    y_ap = nc.dram_tensor("y", [IPC, COUT, H, W], f32, kind="ExternalOutput").ap()

    with tile.TileContext(nc) as tc:
        with (
            tc.tile_pool(name="const", bufs=1) as cp,
            tc.tile_pool(name="x2", bufs=2) as xp,
            tc.tile_pool(name="oimg", bufs=1) as op,
            tc.tile_pool(name="psum", bufs=6, space="PSUM") as pp,
        ):
            w_sb = cp.tile([128, 9 * 128], f32r)
            nc.sync.dma_start(w_sb[:].rearrange("k (t m) -> k t m", t=9),
                              wt_ap.rearrange("t k m -> k t m"))
            bias_sb = cp.tile([128, 1], f32)
            nc.sync.dma_start(bias_sb[:], b_ap[:])

            RB = 28  # rows per input/output DMA block
            for p in range(NPAIR):
                x2 = xp.tile([128, LB], f32r)
                # zero the pad borders (rest is overwritten by the image DMA);
                # memset doesn't accept f32r so bitcast the views to f32
                nc.vector.memset(x2[:, 0:Wp].bitcast(f32), 0.0)          # top pad row
                nc.vector.memset(x2[:, (Hp - 1) * Wp:LB].bitcast(f32), 0.0)  # bottom+tail
                interior = x2[:, Wp:Wp + H * Wp].rearrange("p (h w) -> p h w", w=Wp)
                nc.vector.memset(interior[:, :, 0:1].bitcast(f32), 0.0)  # left pad col
                nc.vector.memset(interior[:, :, Wp - 1:Wp].bitcast(f32), 0.0)  # right pad
                # input row-blocks: fine-grained so matmuls start after block 0
                for rb in range(0, H, RB):
                    for h in range(2):
                        img = 2 * p + h
                        lo = Wp * (1 + rb) + 1
                        dst = x2[64 * h:64 * (h + 1), lo:lo + RB * Wp]
                        dst = dst.rearrange("p (h w) -> p h w", w=Wp)[:, :, 0:W]
                        nc.sync.dma_start(dst, x_ap[img, :, rb:rb + RB, :])

                oimg = op.tile([128, LVAL], f32)
                next_rb = 0
                for c in range(NCHUNK):
                    s = c * CH
                    ps = pp.tile([128, CH], f32)
                    for t in range(9):
                        r, cc = divmod(t, 3)
                        off = r * Wp + cc
                        nc.tensor.matmul(ps[:], w_sb[:, t * 128:(t + 1) * 128],
                                         x2[:, s + off:s + off + CH],
                                         start=(t == 0), stop=(t == 8))
                    ev = min(CH, LVAL - s)
                    nc.vector.tensor_scalar_add(oimg[:, s:s + ev], ps[:, 0:ev],
                                                bias_sb[:])
                    # emit output row-blocks as soon as their rows are evacuated
                    # (ScalarE's DMA queue, so they never block input loads)
                    while next_rb < H and (next_rb + RB) * Wp <= s + ev:
                        for h in range(2):
                            img = 2 * p + h
                            src = oimg[64 * h:64 * (h + 1),
                                       Wp * next_rb:Wp * (next_rb + RB)]
                            src = src.rearrange("p (h w) -> p h w", w=Wp)[:, :, 0:W]
                            nc.scalar.dma_start(y_ap[img, :, next_rb:next_rb + RB, :],
                                                src)
                        next_rb += RB

    nc.compile()
    nc.m = get_hw_module(nc.m)
    return nc


def _get_module():
    if "nc" not in _CACHE:
        _CACHE["nc"] = _build_module()
    return _CACHE["nc"]


def _make_in_maps(x, weight, bias):
    wt = np.zeros((9, 128, 128), np.float32)
    for t in range(9):
        r, cc = divmod(t, 3)
        wT = np.ascontiguousarray(weight[:, :, r, cc].T)  # [cin, cout]
        wt[t, :64, :64] = wT
        wt[t, 64:, 64:] = wT
    bias2 = np.tile(np.asarray(bias, np.float32).reshape(COUT, 1), (2, 1))
    x = np.asarray(x, np.float32)
    return [{"x": np.ascontiguousarray(x[c * IPC:(c + 1) * IPC]),
             "wt": wt, "bias2": bias2} for c in range(N_CORES)]


def _run(in_maps, trace=False):
    from concourse import bass_utils
    nc = _get_module()
    return bass_utils.run_bass_kernel_spmd(
        nc, in_maps, core_ids=list(range(N_CORES)), trace=trace)


def kernel(x, weight, bias):
    res = _run(_make_in_maps(x, weight, bias), trace=False)
    return np.concatenate([res.results[c]["y"] for c in range(N_CORES)], axis=0)


# revision 7
# speedup vs baseline: 1.6084x; 1.0595x over previous
"""Trainium2 Bass kernel for nn_Conv2d (B=32, Cin=Cout=64, H=W=112, 3x3, pad 1).

Strategy:
- Data-parallel: 32 images / 8 cores = 4 images per core; weights/bias replicated.
- Per core: process 2 image-PAIRS. Image A lives on SBUF partitions 0-63,
  image B on partitions 64-127, each zero-padded to 114x114 and flattened.
- Conv = 9 accumulating PE matmuls per 512-pixel chunk: for tap (r,c) the
  stationary lhsT is a 128x128 block-diagonal tile diag(w_rc^T, w_rc^T) so the
  two images convolve independently; the moving rhs is the padded image buffer
  at free-dim offset r*114+c. PSUM (fp32) accumulates all 9 taps.
- Epilogue: VectorE tensor_scalar_add(psum + per-partition bias) -> SBUF staging,
  then one strided DMA per image drops the pad columns.
- dtype float32r: fp32 storage, fast PE mode (~1 cycle/row at N=512),
  measured ~1.5e-4 max relative error end-to-end.
"""
import numpy as np

B, CIN, COUT, H, W = 32, 64, 64, 112, 112
N_CORES = 8
IPC = B // N_CORES          # images per core = 4
NPAIR = IPC // 2            # image pairs per core = 2
Wp = W + 2                  # padded width 114
Hp = H + 2                  # padded height 114
ROWS_PER_CHUNK = 4
CH = ROWS_PER_CHUNK * Wp    # chunk size 456 (row-aligned; fits one PSUM bank)
NCHUNK = H // ROWS_PER_CHUNK     # 28 chunks cover all 112 output rows exactly
LB = Hp * Wp + 256          # padded-image buffer length (+tail margin)

_CACHE = {}


def _build_module():
    import concourse.tile as tile
    from concourse import bacc, mybir
    from concourse.bass_interp import get_hw_module

    f32 = mybir.dt.float32
    f32r = mybir.dt.float32r

    nc = bacc.Bacc("TRN2", target_bir_lowering=False, debug=False,
                   enable_asserts=False, num_devices=N_CORES)
    x_ap = nc.dram_tensor("x", [IPC, CIN, H, W], f32r, kind="ExternalInput").ap()
    wt_ap = nc.dram_tensor("wt", [9, 128, 128], f32r, kind="ExternalInput").ap()
    b_ap = nc.dram_tensor("bias2", [128, 1], f32, kind="ExternalInput").ap()
    y_ap = nc.dram_tensor("y", [IPC, COUT, H, W], f32, kind="ExternalOutput").ap()

    with tile.TileContext(nc) as tc:
        with (
            tc.tile_pool(name="const", bufs=1) as cp,
            tc.tile_pool(name="x2", bufs=2) as xp,
            tc.tile_pool(name="oimg", bufs=1) as op,
            tc.tile_pool(name="psum", bufs=6, space="PSUM") as pp,
        ):
            w_sb = cp.tile([128, 9 * 128], f32r)
            nc.sync.dma_start(w_sb[:].rearrange("k (t m) -> k t m", t=9),
                              wt_ap.rearrange("t k m -> k t m"))
            bias_sb = cp.tile([128, 1], f32)
            nc.sync.dma_start(bias_sb[:], b_ap[:])

            RB = 28  # rows per input/output DMA block
            for p in range(NPAIR):
                x2 = xp.tile([128, LB], f32r)
                # zero the pad borders (rest is overwritten by the image DMA);
                # memset doesn't accept f32r so bitcast the views to f32
                nc.vector.memset(x2[:, 0:Wp].bitcast(f32), 0.0)          # top pad row
                nc.vector.memset(x2[:, (Hp - 1) * Wp:LB].bitcast(f32), 0.0)  # bottom+tail
                interior = x2[:, Wp:Wp + H * Wp].rearrange("p (h w) -> p h w", w=Wp)
                nc.vector.memset(interior[:, :, 0:1].bitcast(f32), 0.0)  # left pad col
                nc.vector.memset(interior[:, :, Wp - 1:Wp].bitcast(f32), 0.0)  # right pad
                # input row-blocks: fine-grained so matmuls start after block 0;
                # alternate Sync/GpSimd DMA queues (one queue's 448B-row rate
                # is only ~180 GB/s; two queues together saturate HBM)
                for bi, rb in enumerate(range(0, H, RB)):
                    for h in range(2):
                        img = 2 * p + h
                        lo = Wp * (1 + rb) + 1
                        dst = x2[64 * h:64 * (h + 1), lo:lo + RB * Wp]
                        dst = dst.rearrange("p (h w) -> p h w", w=Wp)[:, :, 0:W]
                        eng = nc.sync if h == 0 else nc.gpsimd
                        eng.dma_start(dst, x_ap[img, :, rb:rb + RB, :])

                # staging buffer in final HBM layout (pad columns dropped at
                # evacuation) so output DMAs are fully contiguous
                oimg = op.tile([128, H * W], f32)
                next_rb = 0
                for c in range(NCHUNK):
                    s = c * CH
                    ps = pp.tile([128, CH], f32)
                    for t in range(9):
                        r, cc = divmod(t, 3)
                        off = r * Wp + cc
                        nc.tensor.matmul(ps[:], w_sb[:, t * 128:(t + 1) * 128],
                                         x2[:, s + off:s + off + CH],
                                         start=(t == 0), stop=(t == 8))
                    # psum holds ROWS_PER_CHUNK padded rows; keep the 112 valid
                    # columns of each, add bias, write contiguous HBM layout
                    pv = ps[:].rearrange("p (h w) -> p h w", w=Wp)[:, :, 0:W]
                    ov = oimg[:, c * ROWS_PER_CHUNK * W:(c + 1) * ROWS_PER_CHUNK * W]
                    nc.vector.tensor_scalar_add(
                        ov.rearrange("p (h w) -> p h w", w=W), pv, bias_sb[:])
                    # emit output row-blocks as soon as their rows are evacuated
                    # (ScalarE's DMA queue, so they never block input loads)
                    while next_rb < H and (next_rb + RB) <= (c + 1) * ROWS_PER_CHUNK:
                        for h in range(2):
                            img = 2 * p + h
                            src = oimg[64 * h:64 * (h + 1),
                                       W * next_rb:W * (next_rb + RB)]
                            nc.scalar.dma_start(
                                y_ap[img, :, next_rb:next_rb + RB, :],
                                src.rearrange("p (h w) -> p h w", w=W))
                        next_rb += RB

    nc.compile()
    nc.m = get_hw_module(nc.m)
    return nc


def _get_module():
    if "nc" not in _CACHE:
        _CACHE["nc"] = _build_module()
    return _CACHE["nc"]


def _make_in_maps(x, weight, bias):
    wt = np.zeros((9, 128, 128), np.float32)
    for t in range(9):
        r, cc = divmod(t, 3)
        wT = np.ascontiguousarray(weight[:, :, r, cc].T)  # [cin, cout]
        wt[t, :64, :64] = wT
        wt[t, 64:, 64:] = wT
    bias2 = np.tile(np.asarray(bias, np.float32).reshape(COUT, 1), (2, 1))
    x = np.asarray(x, np.float32)
    return [{"x": np.ascontiguousarray(x[c * IPC:(c + 1) * IPC]),
             "wt": wt, "bias2": bias2} for c in range(N_CORES)]


def _run(in_maps, trace=False):
    from concourse import bass_utils
    nc = _get_module()
    return bass_utils.run_bass_kernel_spmd(
        nc, in_maps, core_ids=list(range(N_CORES)), trace=trace)


def kernel(x, weight, bias):
    res = _run(_make_in_maps(x, weight, bias), trace=False)
    return np.concatenate([res.results[c]["y"] for c in range(N_CORES)], axis=0)
